# revision 15
# baseline (speedup 1.0000x reference)
"""Trainium2 Bass kernel for Conv2d: B=16, Cin=Cout=16, H=W=512, k=3, stride=1, pad=1.

Strategy:
  - Data-parallel over batch: 8 cores x 2 images each. Weights/bias replicated.
  - Per core the conv is a sequence of TensorEngine matmuls in an H-Toeplitz
    packing: contraction K = 16 ci x 8 input rows = 128, stationary
    M = 16 co x 6 output rows = 96, moving N = 512 w-pixels. Each chunk of 6
    output rows takes 3 matmuls (one per kw tap, column-shifted rhs)
    accumulating into one PSUM bank; kh lives inside the Toeplitz stationary.
  - fp32r matmuls (PE 1 cycle/col): inputs pre-rounded host-side to the PE's
    fp32r format (12-bit significand, RNE). fp32r needs even PSUM dst
    offset/size, so x is zero-padded to 514 cols and all taps write [0:512].
  - Host-side gathered DRAM layouts:
      xg[b, ci, hi, j, w'] = xpad[b, ci, 6j+hi, w']   (8/6 row duplication)
      yg[b, co, ho, j, w]  -> y[b, co, 6j+ho, w]      (scattered back on host)
    so chunk-major group DMAs read/write multi-chunk contiguous runs per
    partition: G=4 chunks per DMA => ~8 KB packets per SDMA engine instead
    of 2 KB, and 4x fewer DMA instructions.
  - Partition layouts are channel-major (ci*8+hi / co*6+ho) and every DMA's
    DRAM-side outer dim is the 16-entry channel dim -> the HWDGE spreads each
    transfer across all 16 SDMA engines (outer-dim count == engine fan-out).
  - Input DMAs ride the sync HWDGE ring, output DMAs the scalar HWDGE ring.
  - Bias-add + PSUM->SBUF copy on the (otherwise idle) vector engine.
  - Matmuls issue kw-major inside a group (all chunks' kw=1, then kw=0, then
    kw=2) so the stationary weights switch 3x per group instead of 12x; the
    G open PSUM accumulation groups live in distinct banks.
"""

import numpy as np

B, CIN, COUT, H, W = 16, 16, 16, 512, 512
NCORES = 8
BPC = B // NCORES  # images per core
T_OUT, T_IN = 6, 8
KP, MP = T_IN * CIN, T_OUT * COUT  # 128, 96
NCHUNK = (H + T_OUT - 1) // T_OUT  # 86
WPAD = W + 2  # 514 padded cols
GRP = 8  # chunks per DMA group (86 = 10*8 + 6)
SUB = 4  # chunks per PSUM sub-round

DEFAULT_CFG = dict(mm_dtype="fp16", in_dma="sync", out_dma="scalar", grp=8)

_cached = {}


def _groups(grp):
    out = []
    j = 0
    while j < NCHUNK:
        g = min(grp, NCHUNK - j)
        out.append((j, g))
        j += g
    return out


def _build_program(**overrides):
    cfg = dict(DEFAULT_CFG, **overrides)
    key = tuple(sorted(cfg.items()))
    if key in _cached:
        return _cached[key]
    import concourse.bacc as bacc
    import concourse.tile as tile
    import concourse.mybir as mybir

    nc = bacc.Bacc(
        "TRN2",
        target_bir_lowering=False,
        debug=False,
        enable_asserts=False,
        num_devices=NCORES,
    )
    f32 = mybir.dt.float32
    xdt = {"fp32r": mybir.dt.float32r, "fp16": mybir.dt.float16}.get(
        cfg["mm_dtype"], f32
    )
    x = nc.dram_tensor(
        "x", [BPC, CIN, T_IN, NCHUNK, WPAD], xdt, kind="ExternalInput"
    ).ap()
    wt = nc.dram_tensor("wt", [KP, 3 * MP], xdt, kind="ExternalInput").ap()
    bias = nc.dram_tensor("bias", [MP, 1], f32, kind="ExternalInput").ap()
    y = nc.dram_tensor(
        "y", [BPC, COUT, T_OUT, NCHUNK, W], f32, kind="ExternalOutput"
    ).ap()

    in_eng = getattr(nc, cfg["in_dma"])
    out_eng = getattr(nc, cfg["out_dma"])
    grp = cfg.get("grp", GRP)

    with tile.TileContext(nc) as tc:
        with (
            tc.tile_pool(name="consts", bufs=1) as cpool,
            tc.tile_pool(name="xin", bufs=4) as xpool,
            tc.tile_pool(name="psum", bufs=2, space="PSUM") as ppool,
            tc.tile_pool(name="outs", bufs=4) as opool,
        ):
            wt_sb = cpool.tile([KP, 3 * MP], xdt)
            nc.sync.dma_start(wt_sb[:], wt[:])
            bias_sb = cpool.tile([MP, 1], f32)
            nc.sync.dma_start(bias_sb[:], bias[:])

            for b in range(BPC):
                for j0, g in _groups(grp):
                    X = xpool.tile([KP, grp * WPAD], xdt, tag="X")
                    # partition (ci*8+hi) <- g chunks, contiguous per
                    # partition in the gathered DRAM layout
                    in_eng.dma_start(
                        X[:, 0 : g * WPAD],
                        x[b, :, :, j0 : j0 + g, :],
                    )
                    out_sb = opool.tile([MP, grp * W], f32, tag="out")
                    for s0 in range(0, g, SUB):
                        sg = min(SUB, g - s0)
                        pss = [
                            ppool.tile([MP, W], f32, tag=f"ps{k}", name=f"ps{k}")
                            for k in range(sg)
                        ]
                        for i, kw in enumerate((1, 0, 2)):
                            for k in range(sg):
                                gi = s0 + k
                                nc.tensor.matmul(
                                    pss[k][:, :],
                                    wt_sb[:, kw * MP : (kw + 1) * MP],
                                    X[:, gi * WPAD + kw : gi * WPAD + kw + W],
                                    start=(i == 0),
                                    stop=(i == 2),
                                )
                        for k in range(sg):
                            gi = s0 + k
                            nc.vector.tensor_scalar_add(
                                out_sb[:, gi * W : (gi + 1) * W],
                                pss[k][:, :],
                                bias_sb[:, 0:1],
                            )
                    # partition (co*6+ho) -> yg[b, co, ho, j0+gi, :]
                    out_eng.dma_start(
                        y[b, :, :, j0 : j0 + g, :],
                        out_sb[:, 0 : g * W],
                    )
    nc.compile()
    _cached[key] = nc
    return nc


def _rne12(a: np.ndarray) -> np.ndarray:
    """Round fp32 to the PE's fp32r format: 12-bit significand, RNE."""
    u = a.astype(np.float32).view(np.uint32)
    low = u & np.uint32(0xFFF)
    half = np.uint32(0x800)
    rnd = (low > half) | ((low == half) & ((u >> np.uint32(12)) & np.uint32(1)).astype(bool))
    u = (u & np.uint32(0xFFFFF000)) + rnd.astype(np.uint32) * np.uint32(0x1000)
    return u.view(np.float32)


def _toeplitz_weights(weights: np.ndarray) -> np.ndarray:
    """[COUT, CIN, 3, 3] -> [KP, 3*MP] with K index ci*T_IN+hi and M index
    co*T_OUT+ho; lhsT_kw[ci*8+hi, co*6+ho] = W[co, ci, hi-ho, kw] for
    0 <= hi-ho <= 2, else 0. kw blocks side by side."""
    wt = np.zeros((3, CIN, T_IN, COUT, T_OUT), dtype=np.float32)
    for kw in range(3):
        for ho in range(T_OUT):
            for kh in range(3):
                wt[kw, :, ho + kh, :, ho] = weights[:, :, kh, kw].T
    wt2 = wt.reshape(3, KP, MP)
    return np.ascontiguousarray(np.concatenate([wt2[0], wt2[1], wt2[2]], axis=1))


def _make_in_maps(x, weights, biases, mm_dtype="fp32r"):
    wt_packed = _toeplitz_weights(weights)
    bias_vec = np.ascontiguousarray(np.repeat(biases, T_OUT).reshape(MP, 1))
    npdt = np.float32
    if mm_dtype == "fp32r":
        x = _rne12(x)
        wt_packed = _rne12(wt_packed)
    elif mm_dtype == "fp16":
        npdt = np.float16
        x = x.astype(np.float16)
        wt_packed = wt_packed.astype(np.float16)
    # zero-pad to [HP, WPAD] then gather rows: xg[b,ci,hi,j,w] = xp[b,ci,6j+hi,w]
    hp = T_OUT * NCHUNK + 2  # 518
    xp = np.zeros((B, CIN, hp, WPAD), dtype=npdt)
    xp[:, :, 1 : 1 + H, 1 : 1 + W] = x
    rows = np.arange(T_IN)[:, None] + T_OUT * np.arange(NCHUNK)[None, :]  # [8, 86]
    xg = xp[:, :, rows, :]  # [B, CIN, 8, 86, WPAD]
    return [
        {
            "x": np.ascontiguousarray(xg[k * BPC : (k + 1) * BPC]),
            "wt": wt_packed,
            "bias": bias_vec,
        }
        for k in range(NCORES)
    ]


def _gather_output(res_list):
    yg = np.concatenate(res_list, axis=0)  # [B, COUT, 6, NCHUNK, W]
    yfull = yg.transpose(0, 1, 3, 2, 4).reshape(B, COUT, NCHUNK * T_OUT, W)
    return np.ascontiguousarray(yfull[:, :, :H, :])


def kernel(x, weights, biases):
    from concourse import bass_utils

    x = np.ascontiguousarray(np.asarray(x, dtype=np.float32))
    weights = np.asarray(weights, dtype=np.float32)
    biases = np.asarray(biases, dtype=np.float32)

    nc = _build_program()
    in_maps = _make_in_maps(x, weights, biases, DEFAULT_CFG["mm_dtype"])
    res = bass_utils.run_bass_kernel_spmd(nc, in_maps, core_ids=list(range(NCORES)))
    return _gather_output([res.results[k]["y"] for k in range(NCORES)])
